# revision 20
# baseline (speedup 1.0000x reference)
"""Trainium2 Bass kernel for nn_Attention_35734127903400 (v2.2).

Dense transformer attention block:
  xq = LN(x@wq); xk = LN(x@wk); xv = x@wv          (LN over full flattened head dim)
  rope(q, k); GQA self-attention (16 q heads, 8 kv heads, S=2048, full/non-causal)
  gated cross-attention with y (128 tokens); out = (self + tanh(gate)*cross) @ wo

Sharding (8 cores, no collectives): token-sharded. Core c handles batch
b=c//2, sequence half hf=c%2 (1024 q tokens). Each core computes K/V for
its batch's FULL 2048-token sequence (replicated within the pair), Q only
for its local 1024 tokens. LN is over the feature dim so it is core-local.

Everything stays resident in SBUF — no DRAM spill/reload. Stage order:
y proj (startup filler while wq streams) -> Q proj -> K+V proj (fused,
one x pass) -> attention -> output proj (wo streamed per output chunk).

Scheduling specifics, tuned against the TimelineSim cost model:
- proj chunks: the final contraction round runs the single-buffered
  accumulators first and the PSUM->SBUF copies are emitted in the same
  order, so the next chunk's matmuls never wait on a copy.
- LN affine (Pool) + rope (DVE) run per head-half; head transposes (PE,
  bf16 = 1 cycle/row) trail by 1 chunk for half 0 and 2 chunks for
  half 1, hiding the whole post-processing chain.
- attention: softmax denominator via DVE/Pool pairwise-add tree feeding
  5 ones-matmuls spread through the AV stream; cross-attention matmuls
  (dy/oy) slotted where their PSUM bank is free; exp is the pacing
  engine (ACT) and everything else rides in its shadow.
- weight DMAs are per-dc so matmuls start after the first slice; wkv
  prefetches into an untouched right-side SBUF region during Q so the
  KV stage starts without a DMA bubble.
"""

import numpy as np
import ml_dtypes

import concourse.bass as bass
import concourse.mybir as mybir
import concourse.tile as tile
from concourse.bass_utils import run_bass_kernel_spmd
from concourse.masks import make_identity

BF16 = ml_dtypes.bfloat16
F32 = mybir.dt.float32
BF = mybir.dt.bfloat16

P = 128
B, S, D = 4, 2048, 2048
H, KVH = 16, 8
HD = 128
NREP = 2
YL, YD = 128, 1024
EPS = 1e-5
S_LOC = S // 2
DC = D // P          # 16 contraction chunks for D
YDC = YD // P        # 8
TC = S // P          # 16 token chunks (full seq)
TCL = S_LOC // P     # 8 local token chunks
NQ = 512
QCN = S_LOC // NQ    # 2
KVD = KVH * HD       # 1024
SCALE = 1.0 / float(np.sqrt(np.float32(HD)))
AF = mybir.ActivationFunctionType
ALU = mybir.AluOpType

_CACHED = {}
LAST_EXEC_NS = None


def _split_dma_waits(nc, max_waits=1):
    """Hoist excess sync-waits of any instruction onto preceding same-engine
    single-wait NoOps (this build's per-instruction structs have few embedded
    wait slots)."""
    n_split = 0
    for f in nc.m.functions:
        for blk in f.blocks:
            insts = list(blk.instructions)
            out = []
            changed = False
            for ins in insts:
                si = ins.sync_info
                if (si is not None and si.on_wait
                        and len(si.on_wait) > max_waits):
                    waits = list(si.on_wait)
                    for wi, w in enumerate(waits[:-max_waits]):
                        out.append(mybir.InstNoOp(
                            name=f"{ins.name}-wsplit{wi}", engine=ins.engine,
                            sync_info=mybir.SyncInfo(on_wait=[w],
                                                     on_update=[])))
                    ins.sync_info = mybir.SyncInfo(
                        on_wait=waits[-max_waits:],
                        on_update=list(si.on_update))
                    changed = True
                    n_split += 1
                out.append(ins)
            if changed:
                blk.instructions = out
    return n_split


def build_program():
    nc = bass.Bass()

    # ---- I/O (all pre-chunked host-side for >=2KB contiguous runs) ----
    xq_d = nc.declare_dram_parameter("xq", [TCL, P, DC, P], BF, isOutput=False)
    x_d = nc.declare_dram_parameter("x", [TC, P, DC, P], BF, isOutput=False)
    y_d = nc.declare_dram_parameter("y", [P, YDC, YL], BF, isOutput=False)
    wq_d = nc.declare_dram_parameter("wq", [DC, P, D], BF, isOutput=False)
    wkv_d = nc.declare_dram_parameter("wkv", [DC, P, 2 * KVD], BF,
                                      isOutput=False)
    wkvy_d = nc.declare_dram_parameter("wkvy", [YDC, P, 2 * KVD], BF,
                                       isOutput=False)
    wo_d = nc.declare_dram_parameter("wo", [DC, P, DC, P], BF, isOutput=False)
    qw_d = nc.declare_dram_parameter("qw", [D], F32, isOutput=False)
    qb_d = nc.declare_dram_parameter("qb", [D], F32, isOutput=False)
    kw_d = nc.declare_dram_parameter("kw", [KVD], F32, isOutput=False)
    kb_d = nc.declare_dram_parameter("kb", [KVD], F32, isOutput=False)
    kyw_d = nc.declare_dram_parameter("kyw", [KVD], F32, isOutput=False)
    kyb_d = nc.declare_dram_parameter("kyb", [KVD], F32, isOutput=False)
    cosq_d = nc.declare_dram_parameter("cosq", [P, TCL, HD // 2], F32,
                                       isOutput=False)
    sinq_d = nc.declare_dram_parameter("sinq", [P, TCL, HD // 2], F32,
                                       isOutput=False)
    cosk_d = nc.declare_dram_parameter("cosk", [P, TC, HD // 2], F32,
                                       isOutput=False)
    sink_d = nc.declare_dram_parameter("sink", [P, TC, HD // 2], F32,
                                       isOutput=False)
    gates_d = nc.declare_dram_parameter("gates", [H], F32, isOutput=False)
    ymb_d = nc.declare_dram_parameter("ymb", [YL], F32, isOutput=False)
    outT = nc.declare_dram_parameter("outT", [D, S_LOC], F32, isOutput=True)

    with tile.TileContext(nc) as tc:
        from contextlib import ExitStack
        with ExitStack() as ctx:
            cpool = ctx.enter_context(tc.tile_pool(name="consts", bufs=1))
            ident = cpool.tile([P, P], BF)
            make_identity(nc, ident)
            ones_t = cpool.tile([P, P], BF)
            nc.vector.memset(ones_t, 1.0)
            eps_t = cpool.tile([P, 1], F32)
            nc.vector.memset(eps_t, EPS)
            gates_t = cpool.tile([P, H], F32)
            nc.gpsimd.dma_start(
                out=gates_t,
                in_=bass.AP(tensor=gates_d, offset=0, ap=[[0, P], [1, H]]))
            ymb_t = cpool.tile([P, 1], F32)
            nc.gpsimd.dma_start(
                out=ymb_t,
                in_=bass.AP(tensor=ymb_d, offset=0, ap=[[1, P], [0, 1]]))

            def bcast_vec(pool, dram_h, n):
                t = pool.tile([P, n], F32, tag=f"ln_{dram_h.name}", bufs=1)
                nc.gpsimd.dma_start(
                    out=t,
                    in_=bass.AP(tensor=dram_h, offset=0, ap=[[0, P], [1, n]]))
                return t

            def ln_chain(zn, nln, pool, w_t, b_t, pfx):
                """stats (DVE) -> rstd/negmr -> normalize (ACT). Affine is
                applied by the caller (per-half on Pool)."""
                stats = pool.tile([P, nln, 6], F32, tag=f"{pfx}bnstats")
                for i in range(nln):
                    nc.vector.bn_stats(out=stats[:, i, :],
                                       in_=zn[:, i * NQ:(i + 1) * NQ])
                mv = pool.tile([P, 2], F32, tag=f"{pfx}bnaggr")
                nc.vector.bn_aggr(out=mv, in_=stats)
                rstd = pool.tile([P, 1], F32, tag=f"{pfx}rstd")
                nc.scalar.activation(out=rstd, in_=mv[:, 1:2],
                                     func=AF.Sqrt, bias=eps_t, scale=1.0)
                nc.vector.reciprocal(out=rstd, in_=rstd)
                negmr = pool.tile([P, 1], F32, tag=f"{pfx}negmr")
                nc.vector.tensor_scalar(
                    out=negmr, in0=mv[:, 0:1], scalar1=rstd, scalar2=-1.0,
                    op0=ALU.mult, op1=ALU.mult)
                nc.scalar.activation(out=zn, in_=zn, func=AF.Identity,
                                     scale=rstd, bias=negmr)

            # =========================================================
            # Stage Y: y projections -> YKT (LN, no rope), YV.
            # Runs first: its matmuls fill the PE while wq streams in.
            # =========================================================
            yp = ctx.enter_context(tc.tile_pool(name="ypool", bufs=1))
            YKT = yp.tile([P, KVH, YL], BF)
            YV = yp.tile([P, KVH, HD], BF)
            lny = tc.alloc_tile_pool(name="lny", bufs=1)
            wY = tc.alloc_tile_pool(name="wY", bufs=1)
            yt = wY.tile([P, YDC, YL], BF, tag="yt")
            nc.sync.dma_start(out=yt, in_=y_d[:, :, :])
            wy_sb = []
            for g in range(YDC):
                wt = wY.tile([P, 2 * KVD], BF, tag=f"wy{g}", name=f"wy{g}")
                nc.sync.dma_start(out=wt, in_=wkvy_d[g])
                wy_sb.append(wt)
            kyw_t = bcast_vec(wY, kyw_d, KVD)
            kyb_t = bcast_vec(wY, kyb_d, KVD)
            psY = tc.alloc_tile_pool(name="psY", bufs=1, space="PSUM")
            ya = [psY.tile([P, NQ], F32, tag=f"ya{n}", bufs=1, name=f"ya{n}")
                  for n in range(4)]
            for dc in range(YDC):
                for n in range(4):
                    nc.tensor.matmul(
                        ya[n][:], lhsT=yt[:, dc, :],
                        rhs=wy_sb[dc][:, n * NQ:(n + 1) * NQ],
                        start=(dc == 0), stop=(dc == YDC - 1))
            ykn = wY.tile([P, KVD], F32, tag="ykn")
            for n in range(2):
                nc.scalar.copy(out=ykn[:, n * NQ:(n + 1) * NQ], in_=ya[n][:])
                nc.scalar.copy(out=YV[:, 4 * n:4 * (n + 1), :],
                               in_=ya[2 + n][:])
            ln_chain(ykn, 2, wY, kyw_t, kyb_t, "y")
            nc.gpsimd.tensor_mul(out=ykn, in0=ykn, in1=kyw_t)
            nc.gpsimd.tensor_add(out=ykn, in0=ykn, in1=kyb_t)
            ykbf = wY.tile([P, KVH, HD], BF, tag="ykbf")
            nc.vector.tensor_copy(out=ykbf, in_=ykn)
            for hg in range(2):
                tp = psY.tile([P, 4, P], BF, tag="ytr", bufs=2)
                for j in range(4):
                    nc.tensor.transpose(
                        tp[:, j, :], ykbf[:, hg * 4 + j, :], ident)
                nc.scalar.copy(
                    out=YKT[:, hg * 4:(hg + 1) * 4, :], in_=tp)
            psY.release()
            wY.release()
            lny.release()

            qtp = ctx.enter_context(tc.tile_pool(name="qtpool", bufs=1))
            QT = qtp.tile([P, H, S_LOC], BF)

            # =========================================================
            # shared projection-stage machinery
            # =========================================================
            # final-contraction-round matmul order / copy order: the
            # single-buffered accumulators (2, 3) finish and copy first
            ACC_ORDER = [2, 3, 0, 1]

            def proj_stage(nchunks, x_dram, w_tiles, out_heads,
                           w_t, b_t, cos_dram, sin_dram, dst_T, dst_V, stage,
                           xs, preloaded, hooks):
                """One pass over `nchunks` token chunks with 4 accumulators.

                The first `out_heads*HD` features get LN+rope+transpose into
                dst_T; for the KV stage accumulators [2,3] are V, copied raw
                into dst_V[:, chunk, :]. `xs`: caller-owned x-tile pool
                (chunks in `preloaded` were DMA'd by the caller before the
                weight DMAs). `hooks[t]` runs after chunk t's x DMA — used to
                interleave next-stage prefetch DMAs into the SP queue.
                """
                nacc = 4
                csp = tc.alloc_tile_pool(name=f"cs{stage}", bufs=1)
                cs_tiles = {}
                nhalves = (nchunks + 7) // 8

                def load_cs_half(hh):
                    ct = csp.tile([P, 8, HD // 2], F32, tag="ctab",
                                  bufs=nhalves, name=f"ctab{stage}_{hh}")
                    st = csp.tile([P, 8, HD // 2], F32, tag="stab",
                                  bufs=nhalves, name=f"stab{stage}_{hh}")
                    nc.sync.dma_start(out=ct,
                                      in_=cos_dram[:, 8 * hh:8 * hh + 8, :])
                    nc.sync.dma_start(out=st,
                                      in_=sin_dram[:, 8 * hh:8 * hh + 8, :])
                    cs_tiles[hh] = (ct, st)

                load_cs_half(0)
                wk_ = tc.alloc_tile_pool(name=f"work{stage}", bufs=2)
                bfp = tc.alloc_tile_pool(name=f"bf{stage}", bufs=3)
                stp = tc.alloc_tile_pool(name=f"st{stage}", bufs=2)
                rtp = tc.alloc_tile_pool(name=f"rt{stage}", bufs=1)
                psP = tc.alloc_tile_pool(name=f"ps{stage}", bufs=1,
                                         space="PSUM")
                nfeat = out_heads * HD
                nln = nfeat // NQ        # accumulators covered by LN
                oh2 = out_heads // 2     # heads per half
                pending = []             # [(zbf, tok0)]

                def emit_transposes(zbf, tok0):
                    for hg in range(out_heads // 4):
                        h0 = hg * 4
                        tp = psP.tile([P, 4, P], BF, tag="tr", bufs=2)
                        for j in range(4):
                            nc.tensor.transpose(
                                tp[:, j, :], zbf[:, h0 + j, :], ident)
                        nc.scalar.copy(
                            out=dst_T[:, h0:h0 + 4, tok0:tok0 + P],
                            in_=tp)

                def drain(n):
                    for _ in range(n):
                        if pending:
                            emit_transposes(*pending.pop(0))

                for t in range(nchunks):
                    if t in preloaded:
                        xt = preloaded[t]
                    else:
                        xt = xs.tile([P, DC, P], BF, tag="xt",
                                     name=f"xt{stage}_{t}")
                        nc.sync.dma_start(out=xt, in_=x_dram[t])
                    if t in hooks:
                        hooks[t]()
                    if (t % 8 == 6 and t + 2 < nchunks
                            and (t + 2) // 8 not in cs_tiles):
                        load_cs_half((t + 2) // 8)
                    accs = [psP.tile([P, NQ], F32, tag=f"acc{n}",
                                     bufs=(2 if n < 2 else 1),
                                     name=f"acc{n}_{t}")
                            for n in range(nacc)]
                    for dc in range(DC - 1):
                        for n in range(nacc):
                            nc.tensor.matmul(
                                accs[n][:], lhsT=xt[:, dc, :],
                                rhs=w_tiles[dc][:, n * NQ:(n + 1) * NQ],
                                start=(dc == 0), stop=False)
                    for n in ACC_ORDER:
                        nc.tensor.matmul(
                            accs[n][:], lhsT=xt[:, DC - 1, :],
                            rhs=w_tiles[DC - 1][:, n * NQ:(n + 1) * NQ],
                            start=False, stop=True)
                    # PSUM -> SBUF copies, staggered order matching the
                    # final round so the next chunk never waits
                    zn = wk_.tile([P, nfeat], F32, tag="work")
                    for n in ACC_ORDER:
                        if n < nln:
                            nc.scalar.copy(out=zn[:, n * NQ:(n + 1) * NQ],
                                           in_=accs[n][:])
                        elif dst_V is not None:
                            nc.scalar.copy(
                                out=dst_V[:, t, (n - nln) * NQ:
                                          (n - nln + 1) * NQ],
                                in_=accs[n][:])
                    # transposes of chunk t-2 (rope long finished)
                    if t >= 2:
                        drain(1)
                    ln_chain(zn, nln, stp, w_t, b_t, stage)
                    # per-half: affine on Pool, rope on DVE -> bf16
                    zbf = bfp.tile([P, out_heads, HD], BF, tag="zbf")
                    zv = zn.rearrange("p (h f two) -> p h f two",
                                      h=out_heads, two=2)
                    zb = zbf.rearrange("p h (f two) -> p h f two", two=2)
                    ct_t, st_t = cs_tiles[t // 8]
                    shp = (P, oh2, HD // 2)
                    cb = ct_t[:, t % 8, :][:, None, :].to_broadcast(shp)
                    sb = st_t[:, t % 8, :][:, None, :].to_broadcast(shp)
                    for half in range(2):
                        f0 = half * (nfeat // 2)
                        f1 = (half + 1) * (nfeat // 2)
                        nc.gpsimd.tensor_mul(out=zn[:, f0:f1],
                                             in0=zn[:, f0:f1],
                                             in1=w_t[:, f0:f1])
                        nc.gpsimd.tensor_add(out=zn[:, f0:f1],
                                             in0=zn[:, f0:f1],
                                             in1=b_t[:, f0:f1])
                        h0, h1 = half * oh2, (half + 1) * oh2
                        re = zv[:, h0:h1, :, 0]
                        im = zv[:, h0:h1, :, 1]
                        rebf = zb[:, h0:h1, :, 0]
                        imbf = zb[:, h0:h1, :, 1]
                        t1 = rtp.tile([P, oh2, HD // 2], F32, tag="r1")
                        t2 = rtp.tile([P, oh2, HD // 2], F32, tag="r2")
                        nc.vector.tensor_mul(out=t1, in0=re, in1=cb)
                        nc.vector.tensor_mul(out=t2, in0=im, in1=sb)
                        nc.vector.tensor_sub(out=rebf, in0=t1, in1=t2)
                        nc.vector.tensor_mul(out=t1, in0=re, in1=sb)
                        nc.vector.tensor_mul(out=t2, in0=im, in1=cb)
                        nc.vector.tensor_add(out=imbf, in0=t1, in1=t2)
                    pending.append((zbf, t * P))
                while pending:
                    drain(1)
                for pool in (psP, rtp, stp, bfp, wk_, csp):
                    pool.release()

            # =========================================================
            # Stage Q: local-half Q projection
            # =========================================================
            lnq = tc.alloc_tile_pool(name="lnq", bufs=1)
            qw_t = bcast_vec(lnq, qw_d, D)
            qb_t = bcast_vec(lnq, qb_d, D)
            xsQ = tc.alloc_tile_pool(name="xsQ", bufs=2)
            xq0 = xsQ.tile([P, DC, P], BF, tag="xt", name="xtQ_0")
            nc.sync.dma_start(out=xq0, in_=xq_d[0])
            xq1 = xsQ.tile([P, DC, P], BF, tag="xt", name="xtQ_1")
            nc.sync.dma_start(out=xq1, in_=xq_d[1])
            wQ = tc.alloc_tile_pool(name="wQ", bufs=1)
            wq_sb = []
            for g in range(DC):
                wt = wQ.tile([P, D], BF, tag=f"wq{g}", name=f"wq{g}")
                nc.sync.dma_start(out=wt, in_=wq_d[g])
                wq_sb.append(wt)

            # prefetch first half of wkv into untouched right-side SBUF
            wKVa = tc.alloc_tile_pool(name="wKVa", bufs=1, side="right")
            wkv_sb = [None] * DC

            def hook_wkva():
                for g in range(8):
                    wt = wKVa.tile([P, 2 * KVD], BF, tag=f"wkv{g}",
                                   name=f"wkv{g}")
                    nc.sync.dma_start(out=wt, in_=wkv_d[g])
                    wkv_sb[g] = wt

            proj_stage(TCL, xq_d, wq_sb, H, qw_t, qb_t,
                       cosq_d, sinq_d, QT, None, "Q",
                       xsQ, {0: xq0, 1: xq1}, {3: hook_wkva})
            wQ.release()
            xsQ.release()
            lnq.release()

            # =========================================================
            # Stage KV: full-seq K (LN+rope) and V projections, one x pass
            # =========================================================
            ktvp = ctx.enter_context(tc.tile_pool(name="ktvpool", bufs=1))
            KT = ktvp.tile([P, KVH, S], BF)
            Vsb = ktvp.tile([P, TC, KVD], BF)
            lnk = tc.alloc_tile_pool(name="lnk", bufs=1)
            kw_t = bcast_vec(lnk, kw_d, KVD)
            kb_t = bcast_vec(lnk, kb_d, KVD)
            # x tiles ahead of the wkv-second-half DMAs in the SP queue
            xsK = tc.alloc_tile_pool(name="xsK", bufs=2)
            xk0 = xsK.tile([P, DC, P], BF, tag="xt", name="xtK_0")
            nc.sync.dma_start(out=xk0, in_=x_d[0])
            xk1 = xsK.tile([P, DC, P], BF, tag="xt", name="xtK_1")
            nc.sync.dma_start(out=xk1, in_=x_d[1])
            # second wkv half into fresh right-side space: its DMAs have no
            # space-dependency on the Q stage and start immediately
            wKVb = tc.alloc_tile_pool(name="wKVb", bufs=1, side="right")
            for g in range(8, DC):
                wt = wKVb.tile([P, 2 * KVD], BF, tag=f"wkv{g}",
                               name=f"wkv{g}")
                nc.sync.dma_start(out=wt, in_=wkv_d[g])
                wkv_sb[g] = wt
            proj_stage(TC, x_d, wkv_sb, KVH, kw_t, kb_t,
                       cosk_d, sink_d, KT, Vsb, "K",
                       xsK, {0: xk0, 1: xk1}, {})
            xsK.release()
            lnk.release()
            wKVb.release()
            wKVa.release()

            # =========================================================
            # Stage attention: per (head, q-chunk)
            # =========================================================
            mgp = ctx.enter_context(tc.tile_pool(name="merged", bufs=1))
            merged = mgp.tile([P, H, S_LOC], BF)
            wop = tc.alloc_tile_pool(name="wop", bufs=3)
            ep = tc.alloc_tile_pool(name="epool", bufs=5)
            esp = tc.alloc_tile_pool(name="espool", bufs=2)
            eyp = tc.alloc_tile_pool(name="eypool", bufs=2)
            rcp = tc.alloc_tile_pool(name="rcpool", bufs=1)
            psA = tc.alloc_tile_pool(name="psA", bufs=1, space="PSUM")
            outp = tc.alloc_tile_pool(name="outp", bufs=3)
            # prefetch first wo slices during attention
            wo_tiles = {}
            for oc in range(2):
                wo_t = wop.tile([P, DC, P], BF, tag="wo", name=f"wo{oc}")
                nc.sync.dma_start(out=wo_t, in_=wo_d[oc])
                wo_tiles[oc] = wo_t

            def emit_outproj_block(oc, qc):
                """16 matmuls into an o-ring PSUM slot + DVE copy + DMA out.
                ACT-free so it absorbs the exp backlog when interleaved."""
                if oc in wo_tiles:
                    wo_t = wo_tiles.pop(oc)
                else:
                    wo_t = wop.tile([P, DC, P], BF, tag="wo",
                                    name=f"wo{oc}_{qc}")
                    nc.sync.dma_start(out=wo_t, in_=wo_d[oc])
                q0 = qc * NQ
                out_ps = psA.tile([P, NQ], F32, tag="o", bufs=2,
                                  name=f"ops{oc}_{qc}")
                for hc in range(DC):
                    nc.tensor.matmul(
                        out_ps[:], lhsT=wo_t[:, hc, :],
                        rhs=merged[:, hc, q0:q0 + NQ],
                        start=(hc == 0), stop=(hc == DC - 1))
                out_t = outp.tile([P, NQ], F32, tag="outt")
                nc.vector.tensor_copy(out=out_t, in_=out_ps[:])
                nc.sync.dma_start(
                    out=outT[oc * P:(oc + 1) * P, q0:q0 + NQ],
                    in_=out_t)

            # cross scores for iteration idx+1 are issued at g5 of idx so
            # ACT's Ey exp never waits on a cold sy matmul
            def emit_sy(idx):
                qc, h = divmod(idx, H)
                kv = h // NREP
                sy = psA.tile([P, NQ], F32, tag="cr", bufs=1,
                              name=f"sy{idx}")
                nc.tensor.matmul(sy[:], lhsT=YKT[:, kv, :],
                                 rhs=QT[:, h, qc * NQ:qc * NQ + NQ],
                                 start=True, stop=True,
                                 skip_group_check=True)
                Ey = eyp.tile([P, NQ], BF, tag="Ey", name=f"Ey{idx}")
                nc.scalar.activation(out=Ey, in_=sy[:], func=AF.Exp,
                                     scale=SCALE, bias=ymb_t)
                return Ey

            def emit_tail(st):
                """Denominator tail + merge of the previous iteration,
                emitted after the next iteration's first score group so the
                exp backlog never blocks the PE at iteration boundaries."""
                es8_, d_, o_, t1_, h_, q0_ = st
                nc.tensor.matmul(d_[:], lhsT=ones_t, rhs=es8_[:, 2, :],
                                 start=False, stop=False,
                                 skip_group_check=True)
                nc.tensor.matmul(d_[:], lhsT=ones_t, rhs=es8_[:, 6, :],
                                 start=False, stop=False,
                                 skip_group_check=True)
                nc.tensor.matmul(d_[:], lhsT=ones_t, rhs=es8_[:, 7, :],
                                 start=False, stop=True,
                                 skip_group_check=True)
                rec = rcp.tile([P, NQ], F32, tag="rec")
                nc.vector.reciprocal(out=rec, in_=d_[:])
                t0 = rcp.tile([P, NQ], F32, tag="t0")
                nc.vector.tensor_mul(out=t0, in0=o_[:], in1=rec)
                nc.gpsimd.tensor_add(out=merged[:, h_, q0_:q0_ + NQ],
                                     in0=t0, in1=t1_)

            NIT = QCN * H
            Ey = emit_sy(0)
            deferred = None
            for idx in range(NIT):
                qc, h = divmod(idx, H)
                kv = h // NREP
                q0 = qc * NQ
                qt = QT[:, h, q0:q0 + NQ]
                o_ps = psA.tile([P, NQ], F32, tag="o", bufs=2)
                d_ps = None
                es8 = esp.tile([P, 8, NQ], BF, tag="es8")
                for g in range(8):
                    s_ps = psA.tile([P, 2, NQ], F32, tag="s", bufs=2)
                    for j in range(2):
                        kc = 2 * g + j
                        nc.tensor.matmul(
                            s_ps[:, j, :],
                            lhsT=KT[:, kv, kc * P:(kc + 1) * P],
                            rhs=qt, start=True, stop=True,
                            skip_group_check=True)
                    E = ep.tile([P, 2, NQ], BF, tag="E")
                    nc.scalar.activation(out=E, in_=s_ps[:],
                                         func=AF.Exp, scale=SCALE)
                    nc.vector.tensor_add(out=es8[:, g, :],
                                         in0=E[:, 0, :], in1=E[:, 1, :])
                    if g == 0 and deferred is not None:
                        emit_tail(deferred)
                        deferred = None
                    if g in (1, 3, 5):
                        # lvl1 tree add on Pool (in-place into es8[0:3])
                        i = g // 2
                        nc.gpsimd.tensor_add(out=es8[:, i, :],
                                             in0=es8[:, 2 * i, :],
                                             in1=es8[:, 2 * i + 1, :])
                    for j in range(2):
                        kc = 2 * g + j
                        nc.tensor.matmul(
                            o_ps[:],
                            lhsT=Vsb[:, kc, kv * HD:(kv + 1) * HD],
                            rhs=E[:, j, :],
                            start=(kc == 0), stop=(kc == TC - 1),
                            skip_group_check=True)
                    if g == 1:
                        dy = psA.tile([P, NQ], F32, tag="cr", bufs=1,
                                      name=f"dy{idx}")
                        nc.tensor.matmul(dy[:], lhsT=ones_t, rhs=Ey,
                                         start=True, stop=True,
                                         skip_group_check=True)
                        rec_y = rcp.tile([P, NQ], F32, tag="recy")
                        nc.vector.reciprocal(out=rec_y, in_=dy[:])
                    elif g == 3:
                        d_ps = psA.tile([P, NQ], F32, tag="d", bufs=1,
                                        name=f"d{idx}")
                        nc.tensor.matmul(
                            d_ps[:], lhsT=ones_t, rhs=es8[:, 0, :],
                            start=True, stop=False,
                            skip_group_check=True)
                        oy = psA.tile([P, NQ], F32, tag="cr", bufs=1,
                                      name=f"oy{idx}")
                        nc.tensor.matmul(oy[:], lhsT=YV[:, kv, :],
                                         rhs=Ey, start=True, stop=True,
                                         skip_group_check=True)
                        t1 = rcp.tile([P, NQ], F32, tag="t1", bufs=2)
                        nc.vector.scalar_tensor_tensor(
                            out=t1, in0=oy[:],
                            scalar=gates_t[:, h:h + 1],
                            in1=rec_y, op0=ALU.mult, op1=ALU.mult)
                    elif g == 5:
                        nc.tensor.matmul(
                            d_ps[:], lhsT=ones_t, rhs=es8[:, 1, :],
                            start=False, stop=False,
                            skip_group_check=True)
                        if idx + 1 < NIT:
                            next_Ey = emit_sy(idx + 1)
                emit_tail((es8, d_ps, o_ps, t1, h, q0))
                Ey = next_Ey
                # during the second q-chunk pass, interleave the first
                # q-chunk's output projection (ACT-free PE work)
                if qc == 1:
                    emit_outproj_block(h, 0)

            if deferred is not None:
                emit_tail(deferred)
                deferred = None
            # remaining out-proj: second q-chunk
            for oc in range(DC):
                emit_outproj_block(oc, 1)
            psA.release()
            outp.release()
            rcp.release()
            eyp.release()
            esp.release()
            ep.release()
            wop.release()

    _split_dma_waits(nc)
    return nc


def _prep_inputs(x, y, freqs_cos, freqs_sin, y_mask, wq, wk, wv, wk_y, wv_y,
                 wo, q_w, q_b, k_w, k_b, ky_w, ky_b, gate):
    f32 = np.float32

    def chunk_x(xb):
        # [S, D] -> [tc, p, dc, s]: out[t, p, dc, s] = xb[t*128+s, dc*128+p]
        t = xb.shape[0] // P
        return np.ascontiguousarray(
            xb.reshape(t, P, DC, P).transpose(0, 3, 2, 1).astype(BF16))

    def chunk_cs(tab):
        # [S', 64] -> [p, t, f]
        t = tab.shape[0] // P
        return np.ascontiguousarray(
            np.asarray(tab, f32).reshape(t, P, HD // 2).transpose(1, 0, 2))

    wo_f = np.asarray(wo, f32)
    shared = {
        "wq": np.ascontiguousarray(
            np.asarray(wq, f32).astype(BF16).reshape(DC, P, D)),
        "wkv": np.ascontiguousarray(np.concatenate(
            [np.asarray(wk, f32), np.asarray(wv, f32)],
            axis=1).astype(BF16).reshape(DC, P, 2 * KVD)),
        "wkvy": np.ascontiguousarray(np.concatenate(
            [np.asarray(wk_y, f32), np.asarray(wv_y, f32)],
            axis=1).astype(BF16).reshape(YDC, P, 2 * KVD)),
        "wo": np.ascontiguousarray(
            wo_f.reshape(DC, P, DC, P).transpose(2, 1, 0, 3).astype(BF16)),
        "qw": np.ascontiguousarray(np.asarray(q_w, f32)),
        "qb": np.ascontiguousarray(np.asarray(q_b, f32)),
        "kw": np.ascontiguousarray(np.asarray(k_w, f32)),
        "kb": np.ascontiguousarray(np.asarray(k_b, f32)),
        "kyw": np.ascontiguousarray(np.asarray(ky_w, f32)),
        "kyb": np.ascontiguousarray(np.asarray(ky_b, f32)),
        "cosk": chunk_cs(freqs_cos),
        "sink": chunk_cs(freqs_sin),
        "gates": np.ascontiguousarray(np.tanh(np.asarray(gate, f32))),
    }
    per_core = []
    for c in range(8):
        b, hf = c // 2, c % 2
        sl = slice(hf * S_LOC, (hf + 1) * S_LOC)
        xb = np.asarray(x[b], f32)
        m = dict(shared)
        m["x"] = chunk_x(xb)
        m["xq"] = chunk_x(xb[sl])
        m["y"] = np.ascontiguousarray(
            np.asarray(y[b], f32).T.astype(BF16).reshape(YDC, P, YL)
            .transpose(1, 0, 2))
        m["cosq"] = chunk_cs(np.asarray(freqs_cos, f32)[sl])
        m["sinq"] = chunk_cs(np.asarray(freqs_sin, f32)[sl])
        m["ymb"] = np.where(np.asarray(y_mask[b]), 0.0, -1e9).astype(f32)
        per_core.append(m)
    return per_core


def kernel(**inputs):
    if "nc" not in _CACHED:
        _CACHED["nc"] = build_program()
    nc = _CACHED["nc"]
    in_maps = _prep_inputs(
        inputs["x"], inputs["y"], inputs["freqs_cos"], inputs["freqs_sin"],
        inputs["y_mask"], inputs["wq"], inputs["wk"], inputs["wv"],
        inputs["wk_y"], inputs["wv_y"], inputs["wo"], inputs["q_w"],
        inputs["q_b"], inputs["k_w"], inputs["k_b"], inputs["ky_w"],
        inputs["ky_b"], inputs["gate"])
    res = run_bass_kernel_spmd(nc, in_maps, core_ids=list(range(8)))
    global LAST_EXEC_NS
    LAST_EXEC_NS = res.exec_time_ns
    out = np.zeros((B, S, D), np.float32)
    for c in range(8):
        b, hf = c // 2, c % 2
        out[b, hf * S_LOC:(hf + 1) * S_LOC, :] = res.results[c]["outT"].T
    return out


if __name__ == "__main__":
    nc = build_program()
    print("program built OK")
